# revision 22
# baseline (speedup 1.0000x reference)
"""Trainium2 Bass kernel for ContrastiveLabeledLoss.

Reference semantics (C classes, anchor = first row of each class):
    loss = sum_c[count_c >= 2]  mean_{i in c, i != anchor}  || x_i - a_c ||^2

Because the anchor's own distance is zero, the per-class sum over non-anchor
rows equals the sum over ALL rows of the class:
    sum_{i in c} ||x_i - a_c||^2 = S2[c] - 2 * Sv[c] . a_c + n_c * ||a_c||^2
with  Sv[c] = sum_{i in c} x_i   (segment sum,   [C, D])
      S2[c] = sum_{i in c} ||x_i||^2
      n_c   = count of class c
      a_c   = x[argmin_{i in c} i]    (global first occurrence).

Device strategy (data parallel over 8 NeuronCores, rows sharded along N):
  * Each core streams its [32768, 256] fp32 shard once from HBM (memory bound).
  * Per 128-row subtile: a one-hot [128, C] label mask is built on DVE and used
    as the stationary operand of an fp32r matmul whose moving operand is
    [X_sub | X_sub^2] (512 cols), accumulating [C, 512] = [Sv | Sv2] in PSUM.
  * Per-class counts and the per-class min row index are accumulated on DVE.
  * Epilogue: each core gathers its local first-occurrence rows with an
    indirect DMA, then ONE 8-core AllGather shares (stats, count, min-index,
    candidate anchor rows).  Every core reduces to the identical scalar loss.
"""

import sys
from contextlib import ExitStack

import numpy as np

sys.path.insert(0, "/opt/trn_rl_repo")

import concourse.bass as bass  # noqa: E402
import concourse.tile as tile  # noqa: E402
from concourse import bacc, mybir  # noqa: E402
from concourse.bass_utils import run_bass_kernel_spmd  # noqa: E402
from concourse.masks import make_identity  # noqa: E402

N_CORES = 8
N, D, C = 262144, 256, 64
NL = N // N_CORES              # rows per core = 32768
SUBT = NL // 128               # 128-row subtiles per core = 256
T = 32                         # subtiles per main-loop iteration
N_ITERS = SUBT // T            # 8
BIG = 16384.0                  # > SUBT, power of two (exact in fp32)
PAYW = 772                     # payload row: 512 stats | 1 cnt | 1 cand | 256 rows | 2 pad

F32 = mybir.dt.float32
F32R = mybir.dt.float32r
X_AXIS = mybir.AxisListType.X
OP = mybir.AluOpType


def _build_program():
    nc = bacc.Bacc(
        "TRN2", target_bir_lowering=False, debug=False, num_devices=N_CORES
    )

    # f32r view of plain fp32 bytes — keeps the walrus BIR verifier happy about
    # "rounded to FP32r" producers feeding the f32r matmuls (PE rounds anyway).
    x = nc.dram_tensor("x", [NL, D], F32R, kind="ExternalInput")
    labf = nc.dram_tensor("labf", [128, SUBT], F32, kind="ExternalInput")
    corebase = nc.dram_tensor("corebase", [128, 1], F32, kind="ExternalInput")
    loss_out = nc.dram_tensor("loss", [1, 1], F32, kind="ExternalOutput")

    with tile.TileContext(nc) as tc, ExitStack() as ctx:
        _kernel_body(ctx, tc, x, labf, corebase, loss_out)

    nc.compile()
    return nc


def _kernel_body(ctx, tc, x, labf, corebase, loss_out):
    nc = tc.nc

    const_pool = ctx.enter_context(tc.tile_pool(name="const", bufs=1))
    combo_pool = ctx.enter_context(tc.tile_pool(name="combo", bufs=2))
    work_pool = ctx.enter_context(tc.tile_pool(name="work", bufs=2))
    acc_pool = ctx.enter_context(tc.tile_pool(name="acc", bufs=1))
    psum_pool = ctx.enter_context(tc.tile_pool(name="psum", bufs=1, space="PSUM"))
    dram_pool = ctx.enter_context(tc.tile_pool(name="dram", bufs=1, space="DRAM"))

    # ---- constants -------------------------------------------------------
    labels_sb = const_pool.tile([128, SUBT], F32)
    nc.sync.dma_start(labels_sb[:], labf[:])

    cb_sb = const_pool.tile([128, 1], F32)
    nc.sync.dma_start(cb_sb[:], corebase[:])

    iota_c = const_pool.tile([128, C], F32)
    nc.gpsimd.iota(
        iota_c[:], pattern=[[1, C]], base=0, channel_multiplier=0,
        allow_small_or_imprecise_dtypes=True,
    )
    # tplus[p, t] = t + BIG
    tplus = const_pool.tile([128, SUBT], F32)
    nc.gpsimd.iota(
        tplus[:], pattern=[[1, SUBT]], base=int(BIG), channel_multiplier=0,
        allow_small_or_imprecise_dtypes=True,
    )
    iota_p = const_pool.tile([128, 1], F32)
    nc.gpsimd.iota(
        iota_p[:], pattern=[[0, 1]], base=0, channel_multiplier=1,
        allow_small_or_imprecise_dtypes=True,
    )
    ident = const_pool.tile([128, 128], F32)
    make_identity(nc, ident[:])
    ones_c = const_pool.tile([C, 1], F32)
    nc.vector.memset(ones_c[:], 1.0)

    # ---- accumulators ----------------------------------------------------
    stats_ps = psum_pool.tile([C, 512], F32, tag="stats")
    accmin = acc_pool.tile([128, C], F32)
    nc.vector.memset(accmin[:], 4.0 * BIG)
    acccnt = acc_pool.tile([128, C], F32)
    nc.vector.memset(acccnt[:], 0.0)

    # DRAM view of x grouped by subtile: x3[p, t, d] = x[t*128 + p, d]
    x3 = x[:].rearrange("(t p) d -> p t d", p=128)

    import os
    n_iters_run = int(os.environ.get("KERNEL_ITERS", N_ITERS))

    # ---- main streaming loop --------------------------------------------
    for i in range(n_iters_run):
        combo = combo_pool.tile([128, 2 * T * D], F32R, tag="combo")
        xpart = combo[:, 0 : T * D]
        sqpart = combo[:, T * D : 2 * T * D]

        nc.sync.dma_start(
            xpart.rearrange("p (t d) -> p t d", d=D), x3[:, i * T : (i + 1) * T, :]
        )
        nc.scalar.activation(
            sqpart, xpart.bitcast(F32), mybir.ActivationFunctionType.Square
        )

        # one-hot [p, (t, c)] = (labels[p, iT+t] == c)
        oh = work_pool.tile([128, T * C], F32R, tag="oh")
        oh3 = oh.rearrange("p (t c) -> p t c", c=C)
        lab_b = labels_sb[:, i * T : (i + 1) * T].to_broadcast([128, T, C])
        ic = iota_c[:]
        iota_b = bass.AP(ic.tensor, ic.offset, [ic.ap[0], [0, T], [1, C]])
        nc.vector.tensor_tensor(out=oh3, in0=lab_b, in1=iota_b, op=OP.is_equal)

        # candidate row index: match -> t, no match -> t + BIG
        scr = work_pool.tile([128, T * C], F32, tag="scr")
        scr3 = scr.rearrange("p (t c) -> p t c", c=C)
        tplus_b = tplus[:, i * T : (i + 1) * T].to_broadcast([128, T, C])
        nc.vector.scalar_tensor_tensor(
            out=scr3, in0=oh3.bitcast(F32), scalar=-BIG, in1=tplus_b,
            op0=OP.mult, op1=OP.add,
        )
        red = work_pool.tile([128, C], F32, tag="red")
        nc.vector.tensor_reduce(
            out=red[:], in_=scr.rearrange("p (t c) -> p c t", c=C),
            axis=X_AXIS, op=OP.min,
        )
        nc.vector.tensor_tensor(out=accmin[:], in0=accmin[:], in1=red[:], op=OP.min)

        cntred = work_pool.tile([128, C], F32, tag="cntred")
        nc.vector.tensor_reduce(
            out=cntred[:], in_=oh.bitcast(F32).rearrange("p (t c) -> p c t", c=C),
            axis=X_AXIS, op=OP.add,
        )
        nc.vector.tensor_tensor(
            out=acccnt[:], in0=acccnt[:], in1=cntred[:], op=OP.add
        )

        # stats matmuls: [C, 512] += onehot_t.T @ [X_t | X_t^2]
        combo2 = combo.rearrange("p (s e) -> p s e", s=2)
        for t in range(T):
            g = i * T + t
            rhs = combo2[:, :, t * D : (t + 1) * D]
            nc.tensor.matmul(
                out=stats_ps[:],
                lhsT=oh[:, t * C : (t + 1) * C],
                rhs=rhs,
                start=(g == 0),
                stop=(g == n_iters_run * T - 1),
            )

    stage = os.environ.get("KERNEL_STAGE", "f")

    def bail(src_ap):
        loss_sb0 = acc_pool.tile([1, 1], F32, name=f"bail_{stage}")
        nc.vector.tensor_copy(loss_sb0[:], src_ap)
        nc.sync.dma_start(loss_out[:], loss_sb0[:])

    # ---- local epilogue --------------------------------------------------
    # joint[:, 0:C]  = accmin * 128 + p   (linear local row index per class)
    # joint[:, C:2C] = per-partition counts
    if stage == "a":
        return bail(stats_ps[0:1, 0:1])

    joint = acc_pool.tile([128, 2 * C], F32)
    nc.vector.tensor_scalar(
        out=joint[:, 0:C], in0=accmin[:], scalar1=128.0, scalar2=iota_p[:, 0:1],
        op0=OP.mult, op1=OP.add,
    )
    nc.vector.tensor_copy(joint[:, C : 2 * C], acccnt[:])

    tp_ps = psum_pool.tile([128, 128], F32, tag="tp")
    nc.tensor.transpose(out=tp_ps[:], in_=joint[:], identity=ident[:])

    redmin = acc_pool.tile([C, 1], F32)   # local min linear row (or sentinel >= BIG*128)
    nc.vector.tensor_reduce(out=redmin[:], in_=tp_ps[0:C, :], axis=X_AXIS, op=OP.min)
    cnt_loc = acc_pool.tile([C, 1], F32)
    nc.vector.tensor_reduce(
        out=cnt_loc[:], in_=tp_ps[C : 2 * C, :], axis=X_AXIS, op=OP.add
    )

    # global candidate index (sentinel survives; distinct across cores)
    cand = acc_pool.tile([C, 1], F32)
    nc.vector.tensor_scalar(
        out=cand[:], in0=redmin[:], scalar1=cb_sb[0:C, 0:1], scalar2=None, op0=OP.add
    )

    if stage == "b":
        return bail(cand[0:1, 0:1])

    # gather local candidate anchor rows (clamped; garbage masked later)
    rowf = acc_pool.tile([C, 1], F32)
    nc.vector.tensor_scalar(
        out=rowf[:], in0=redmin[:], scalar1=float(NL - 1), scalar2=None, op0=OP.min
    )
    rowi = acc_pool.tile([C, 1], mybir.dt.int32)
    nc.vector.tensor_copy(rowi[:], rowf[:])
    rows = acc_pool.tile([C, D], F32R)
    nc.gpsimd.indirect_dma_start(
        out=rows[:],
        out_offset=None,
        in_=x[:],
        in_offset=bass.IndirectOffsetOnAxis(ap=rowi[:, 0:1], axis=0),
    )

    if stage == "c":
        return bail(rows.bitcast(F32)[0:1, 0:1])

    # ---- share everything with one AllGather -----------------------------
    pay = acc_pool.tile([C, PAYW], F32)
    nc.vector.tensor_copy(pay[:, 0:512], stats_ps[:])
    nc.vector.tensor_copy(pay[:, 512:513], cnt_loc[:])
    nc.vector.tensor_copy(pay[:, 513:514], cand[:])
    nc.vector.tensor_copy(pay[:, 514:770], rows.bitcast(F32))
    nc.vector.memset(pay[:, 770:772], 0.0)

    cc_in = dram_pool.tile([C, PAYW], F32)
    cc_out = dram_pool.tile([N_CORES * C, PAYW], F32, addr_space="Shared")
    nc.sync.dma_start(cc_in[:], pay[:])
    nc.gpsimd.collective_compute(
        "AllGather",
        OP.bypass,
        replica_groups=[list(range(N_CORES))],
        ins=[cc_in[:]],
        outs=[cc_out[:]],
    )

    gath = acc_pool.tile([C, N_CORES * PAYW], F32)
    nc.sync.dma_start(
        gath[:].rearrange("p (j w) -> p j w", j=N_CORES),
        cc_out[:].rearrange("(j p) w -> p j w", p=C),
    )
    g3 = gath[:].rearrange("p (j w) -> p j w", j=N_CORES)

    if stage == "d":
        return bail(gath[0:1, 0:1])

    # global min over cores
    gmin = acc_pool.tile([C, 1], F32)
    nc.vector.tensor_reduce(
        out=gmin[:], in_=gath[:, 513 : N_CORES * PAYW : PAYW], axis=X_AXIS, op=OP.min,
    )
    # total stats / counts over cores (reduce over trailing j via AP reorder)
    stats = acc_pool.tile([C, 512], F32)
    nc.vector.tensor_reduce(
        out=stats[:],
        in_=gath[:].rearrange("p (j w) -> p w j", j=N_CORES)[:, 0:512, :],
        axis=X_AXIS, op=OP.add,
    )
    cnt = acc_pool.tile([C, 1], F32)
    nc.vector.tensor_reduce(
        out=cnt[:], in_=gath[:, 512 : N_CORES * PAYW : PAYW], axis=X_AXIS, op=OP.add
    )

    # anchors = sum_j (cand_j == gmin) * rows_j
    anchors = acc_pool.tile([C, D], F32)
    nc.vector.memset(anchors[:], 0.0)
    eq = acc_pool.tile([C, N_CORES], F32)
    for j in range(N_CORES):
        nc.vector.tensor_tensor(
            out=eq[:, j : j + 1], in0=g3[:, j, 513:514], in1=gmin[:], op=OP.is_equal
        )
        nc.vector.scalar_tensor_tensor(
            out=anchors[:], in0=g3[:, j, 514:770], scalar=eq[:, j : j + 1],
            in1=anchors[:], op0=OP.mult, op1=OP.add,
        )

    if stage == "e":
        return bail(anchors[0:1, 0:1])

    # ---- final scalar ----------------------------------------------------
    scr_a = acc_pool.tile([C, D], F32)
    sva = acc_pool.tile([C, 1], F32)
    nc.vector.tensor_tensor(
        out=scr_a[:], in0=stats[:, 0:D], in1=anchors[:], op=OP.mult
    )
    nc.vector.tensor_reduce(out=sva[:], in_=scr_a[:], axis=X_AXIS, op=OP.add)
    scr_b = acc_pool.tile([C, D], F32)
    a2 = acc_pool.tile([C, 1], F32)
    nc.vector.tensor_tensor(
        out=scr_b[:], in0=anchors[:], in1=anchors[:], op=OP.mult
    )
    nc.vector.tensor_reduce(out=a2[:], in_=scr_b[:], axis=X_AXIS, op=OP.add)
    s2 = acc_pool.tile([C, 1], F32)
    nc.vector.tensor_reduce(out=s2[:], in_=stats[:, D : 2 * D], axis=X_AXIS, op=OP.add)

    t1 = acc_pool.tile([C, 1], F32)
    nc.vector.tensor_tensor(out=t1[:], in0=cnt[:], in1=a2[:], op=OP.mult)
    t2 = acc_pool.tile([C, 1], F32)
    nc.vector.scalar_tensor_tensor(
        out=t2[:], in0=sva[:], scalar=-2.0, in1=s2[:], op0=OP.mult, op1=OP.add
    )
    pc = acc_pool.tile([C, 1], F32)
    nc.vector.tensor_tensor(out=pc[:], in0=t2[:], in1=t1[:], op=OP.add)

    den = acc_pool.tile([C, 1], F32)
    nc.vector.tensor_scalar(
        out=den[:], in0=cnt[:], scalar1=1.0, scalar2=1.0, op0=OP.subtract, op1=OP.max
    )
    rec = acc_pool.tile([C, 1], F32)
    nc.vector.reciprocal(rec[:], den[:])
    val = acc_pool.tile([C, 1], F32)
    nc.vector.tensor_scalar(
        out=val[:], in0=cnt[:], scalar1=2.0, scalar2=None, op0=OP.is_ge
    )

    lc = acc_pool.tile([C, 1], F32)
    nc.vector.tensor_tensor(out=lc[:], in0=pc[:], in1=rec[:], op=OP.mult)
    nc.vector.tensor_tensor(out=lc[:], in0=lc[:], in1=val[:], op=OP.mult)

    # sum over the partition dim via fp32 PE transpose + free-dim reduce
    loss_ps = psum_pool.tile([1, C], F32, tag="loss")
    nc.tensor.transpose(out=loss_ps[:], in_=lc[:], identity=ident[0:C, 0:C])
    loss_sb = acc_pool.tile([1, 1], F32)
    nc.vector.tensor_reduce(out=loss_sb[:], in_=loss_ps[:], axis=X_AXIS, op=OP.add)
    nc.sync.dma_start(loss_out[:], loss_sb[:])


_PROGRAM = None


def _get_program():
    global _PROGRAM
    if _PROGRAM is None:
        _PROGRAM = _build_program()
    return _PROGRAM


def make_in_maps(outputs: np.ndarray, labels: np.ndarray):
    in_maps = []
    for k in range(N_CORES):
        xk = np.ascontiguousarray(outputs[k * NL : (k + 1) * NL]).astype(
            np.float32, copy=False
        )
        lk = np.ascontiguousarray(
            labels[k * NL : (k + 1) * NL].astype(np.float32).reshape(SUBT, 128).T
        )
        in_maps.append(
            {
                "x": xk,
                "labf": lk,
                "corebase": np.full((128, 1), k * NL, np.float32),
            }
        )
    return in_maps


def kernel(outputs: np.ndarray, labels: np.ndarray) -> np.ndarray:
    nc = _get_program()
    in_maps = make_in_maps(np.asarray(outputs), np.asarray(labels))
    res = run_bass_kernel_spmd(nc, in_maps, core_ids=list(range(N_CORES)))
    return np.asarray(res.results[0]["loss"], dtype=np.float32).reshape(1)
